# revision 3
# baseline (speedup 1.0000x reference)
"""Trainium2 Bass kernel for nn_CausalSelfAttention_56925496541402.

Sliding-window (1024) causal self-attention with rotary embedding,
rms-norm on q/k, and a value-embedding (VE) sigmoid gate. B=1, T=4096,
8 heads x 128 head_dim, n_embd=1024.

Sharding: one head per NeuronCore (8 cores). Each core computes its
head's q/k/v projections, rope+rmsnorm, windowed attention, and its
head's slice of the output projection; the host sums the 8 partial
[4096,1024] outputs (row-block contraction of c_proj).

Layouts per core (SBUF):
  qT/kT: [head_dim=128 part, T free]; rope is done with partition-
         shifted DVE operands (no rotate copies); rms partition-sum via
         all-ones matmul; rsqrt via the Abs_reciprocal_sqrt ACT table.
  v:     [T part (128-chunks), head_dim free] (PE-transposed) so the
         PV matmul consumes exp(S)^T directly -- no P transposes.
  S^T:   [j=key part, i=query free] tiles; softmax denominator via
         all-ones matmul + reciprocal_approx_fast; masks multiplicative
         post-exp (two shift-invariant triangular families).

fp16 data path (inputs, weights, attn weights, partial outputs; host
sums partials in f32); matmul accumulation and softmax stats in f32.
exp(S*scale - 4) keeps attention weights inside fp16 range. ACT table
sets: sigmoid -> abs_reciprocal_sqrt_and_small -> exp (3 loads total).
"""
import sys
sys.path.insert(0, "/opt/trn_rl_repo")
import math
import numpy as np

T = 4096
TB = 512           # t-block width
NBLK = T // TB
D = 128            # head dim
C = 1024           # n_embd
NCO = C // 128     # embed chunks
WIN = 1024
NCORES = 8
SCALE = 1.0 / math.sqrt(D)
EXP_BIAS = -4.0    # exp(S*scale - 4): fp16-safe range, cancels in normalize

_prog_cache = {}
_last_in_maps = None


def _chunk_list(b):
    """Key chunks for query block b (i0=512b): (j0, mask_idx, lo, hi).

    [lo, hi) is the computed query range (the chunk's visible window);
    the mask multiply is applied on the 128-wide triangle boundary
    [mlo, mlo+128) inside it. The first chunk covers [0, 512) so its
    start=True matmul initializes every psum column.
    mask m<4 : low window edge, visible iff ii < jj + 128*m
    mask m>=4: causal edge,     visible iff ii >= jj + 128*(m-4)
    """
    i0 = TB * b
    out = []
    for c in range(4):           # full chunks (emitted first)
        j0 = i0 - 512 + 128 * c
        if j0 >= 0:
            out.append((j0, None, 0, 512))
    for c in range(4):           # causal chunks: visible i in [128c, 512)
        j0 = i0 + 128 * c
        out.append((j0, 4 + c, 128 * c, 512))
    for c in range(4):           # low-edge chunks: visible i in [0, 128c+128)
        j0 = i0 - 1024 + 128 * c
        if j0 >= 0:
            out.append((j0, c, 0, 128 * (c + 1)))
    if b == 0:
        # no full chunks: widest causal chunk (c=0, [0,512)) is already first
        assert out[0][2] == 0 and out[0][3] == 512
    return out


def _build_program(nreps=1):
    import concourse.bass as bass
    import concourse.mybir as mybir
    import concourse.tile as tile
    from concourse import bacc
    from concourse.masks import make_identity

    F32 = mybir.dt.float32
    F16 = mybir.dt.float16
    AF = mybir.ActivationFunctionType
    MUL = mybir.AluOpType.mult
    ADD = mybir.AluOpType.add
    ts = bass.ts

    nc = bacc.Bacc("TRN2", target_bir_lowering=False, debug=False,
                   enable_asserts=True, num_devices=1)

    # x_pre[p, co*T + t] = x[t, co*128+p]: per-partition contiguous lines
    xT = nc.dram_tensor("xT", [128, NCO * T], F16, kind="ExternalInput").ap()
    cc_d = nc.dram_tensor("cc", [D, T], F16, kind="ExternalInput").ap()
    ss_d = nc.dram_tensor("ssw", [D, T], F16, kind="ExternalInput").ap()
    veT_d = nc.dram_tensor("veT", [D, T], F16, kind="ExternalInput").ap()
    # w_pre[p, co*128 + d] = w[co*128+p, d]: contiguous per-partition lines
    wq_d = nc.dram_tensor("wq", [128, C], F16, kind="ExternalInput").ap()
    wk_d = nc.dram_tensor("wk", [128, C], F16, kind="ExternalInput").ap()
    wv_d = nc.dram_tensor("wv", [128, C], F16, kind="ExternalInput").ap()
    wp_d = nc.dram_tensor("wp", [D, C], F16, kind="ExternalInput").ap()
    wg_d = nc.dram_tensor("wg", [32, 128], F16, kind="ExternalInput").ap()
    mk_d = nc.dram_tensor("masks", [8, 128, 512], F16, kind="ExternalInput").ap()
    on_d = nc.dram_tensor("ones", [128, 128], F16, kind="ExternalInput").ap()
    out_d = nc.dram_tensor("out", [T, C], F16, kind="ExternalOutput").ap()

    xT3 = xT.rearrange("p (co t) -> p co t", co=NCO)

    with tile.TileContext(nc) as tc:
        with tc.tile_pool(name="const", bufs=1) as cst:
            w_sbs = []
            for wd, nm in ((wq_d, "wq"), (wk_d, "wk"), (wv_d, "wv")):
                w_sb = cst.tile([128, NCO, D], F16, tag=f"w{nm}")
                nc.sync.dma_start(w_sb[:], wd.rearrange("p (co d) -> p co d",
                                                        co=NCO))
                w_sbs.append(w_sb)
            wq_sb, wk_sb, wv_sb = w_sbs
            wp_sb = cst.tile([128, C], F16, tag="wp")
            wg_sb = cst.tile([32, 128], F16, tag="wg")
            nc.sync.dma_start(wg_sb[:], wg_d)
            mk_sb = cst.tile([128, 8, 512], F16, tag="mk")
            on_sb = cst.tile([128, 128], F16, tag="on")
            nc.sync.dma_start(on_sb[:], on_d)
            ident = cst.tile([128, 128], F16, tag="ident")
            make_identity(nc, ident[:])
            eps = cst.tile([128, 1], F32, tag="eps")
            nc.gpsimd.memset(eps[:], 1e-6)
            eb = cst.tile([128, 1], F32, tag="eb")
            nc.gpsimd.memset(eb[:], EXP_BIAS)
            gate = cst.tile([128, T], F16, tag="gate")
            qTn = cst.tile([128, T], F16, tag="qTn")
            kTn = cst.tile([128, T], F16, tag="kTn")
            vsl = cst.tile([128, T // 128, D], F16, tag="vsl")
            cc_sb = cst.tile([128, T], F16, tag="cc")
            ss_sb = cst.tile([128, T], F16, tag="ssw")
            vet = cst.tile([128, T], F16, tag="ve")

            for _rep in range(nreps):
                # ---- phase 0: VE gate sigmoid (own ACT table set) ----
                with tc.tile_pool(name="p0", bufs=2) as p0, \
                     tc.tile_pool(name="p0ps", bufs=2, space="PSUM") as p0ps:
                    x32 = p0.tile([32, T], F16, tag="x32")
                    nc.sync.dma_start(x32[:], xT3[0:32, 0, :])
                    for tb in range(NBLK):
                        sl = ts(tb, TB)
                        gp = p0ps.tile([128, TB], F32, tag="gps")
                        nc.tensor.matmul(gp[:], wg_sb[:], x32[:, sl],
                                         start=True, stop=True)
                        nc.scalar.activation(gate[:, sl], gp[:], AF.Sigmoid)

                # ---- phase 1: q/k/v projections, rope+rmsnorm, v gate+transpose ----
                # ACT funcs here: Abs_reciprocal_sqrt (set 15); Copy (any set)
                # sumsq matmuls are emitted after all three projections so the
                # PE never stalls on the DVE rope chain.
                with tc.tile_pool(name="xp", bufs=3) as xp, \
                     tc.tile_pool(name="sc1", bufs=4) as sc, \
                     tc.tile_pool(name="pps", bufs=4, space="PSUM") as pps, \
                     tc.tile_pool(name="sqps", bufs=2, space="PSUM") as sqps, \
                     tc.tile_pool(name="tps", bufs=2, space="PSUM") as tps:
                    x_sb2 = None
                    for tb in range(NBLK):
                        sl = ts(tb, TB)
                        if tb % 2 == 0:
                            x_sb2 = xp.tile([128, NCO, 2 * TB], F16, tag="x")
                            nc.sync.dma_start(x_sb2[:],
                                              xT3[:, :, ts(tb // 2, 2 * TB)])
                        if tb == 0 and _rep == 0:
                            # emitted after the first x block so the first
                            # projection matmuls aren't starved by table DMAs
                            nc.sync.dma_start(cc_sb[:], cc_d)
                            nc.sync.dma_start(ss_sb[:], ss_d)
                            nc.sync.dma_start(vet[:], veT_d)
                        x_sb = x_sb2[:, :, ts(tb % 2, TB)]
                        sq_tiles = []
                        for w_sb in (wq_sb, wk_sb):
                            up = pps.tile([128, TB], F32, tag="proj")
                            for co in range(NCO):
                                nc.tensor.matmul(up[:], w_sb[:, co, :],
                                                 x_sb[:, co, :],
                                                 start=(co == 0), stop=(co == NCO - 1))
                            u16 = sc.tile([128, TB], F16, tag="u16")
                            nc.scalar.copy(u16[:], up[:])
                            # rope: p[d] = u[d]*ssw[d]; y = u*cc + swap64(p)
                            t1 = sc.tile([128, TB], F16, tag="t1")
                            nc.vector.tensor_tensor(t1[:], u16[:], cc_sb[:, sl], MUL)
                            p = sc.tile([128, TB], F16, tag="p")
                            nc.vector.tensor_tensor(p[:], u16[:], ss_sb[:, sl], MUL)
                            pr = sc.tile([128, TB], F16, tag="pr")
                            nc.vector.tensor_copy(pr[0:64, :], p[64:128, :])
                            nc.vector.tensor_copy(pr[64:128, :], p[0:64, :])
                            y = sc.tile([128, TB], F16, tag="y")
                            nc.vector.tensor_tensor(y[:], t1[:], pr[:], ADD)
                            sq = sc.tile([128, TB], F16, tag="sq")
                            nc.vector.tensor_tensor(sq[:], y[:], y[:], MUL)
                            sq_tiles.append((sq, y))
                        # v = x@wv + sigmoid(gate)*(2*ve)
                        vp = pps.tile([128, TB], F32, tag="proj")
                        for co in range(NCO):
                            nc.tensor.matmul(vp[:], wv_sb[:, co, :], x_sb[:, co, :],
                                             start=(co == 0), stop=(co == NCO - 1))
                        tmp = sc.tile([128, TB], F16, tag="vtmp")
                        nc.vector.tensor_tensor(tmp[:], gate[:, sl], vet[:, sl], MUL)
                        vT = sc.tile([128, TB], F16, tag="vT")
                        nc.vector.tensor_tensor(vT[:], vp[:], tmp[:], ADD)
                        # rms-norm scale (partition-axis sum via all-ones matmul)
                        for (sq, y), slab in zip(sq_tiles, (qTn, kTn)):
                            sp = sqps.tile([128, TB], F32, tag="sumsq")
                            nc.tensor.matmul(sp[:], on_sb[:], sq[:],
                                             start=True, stop=True)
                            rs = sc.tile([128, TB], F16, tag="rs")
                            nc.scalar.activation(rs[:], sp[:],
                                                 AF.Abs_reciprocal_sqrt,
                                                 scale=1.0 / D, bias=eps[:])
                            nc.vector.tensor_tensor(slab[:, sl], y[:], rs[:], MUL)
                        for kk in range(4):
                            tp = tps.tile([128, 128], F16, tag="tp")
                            nc.tensor.transpose(tp[:], vT[:, ts(kk, 128)], ident[:])
                            (nc.scalar.copy if kk % 2 == 0 else
                             nc.vector.tensor_copy)(vsl[:, 4 * tb + kk, :], tp[:])

                # ---- phase 2: windowed attention + output projection ----
                # ACT funcs here: Exp only (exp_and_others). The S matmul for
                # chunk c+2 is emitted before denom/PV of chunk c so the PE
                # never stalls on the ACT exp chain.
                with tc.tile_pool(name="ptp", bufs=10) as ptp, \
                     tc.tile_pool(name="sc2", bufs=4) as sc2, \
                     tc.tile_pool(name="outp", bufs=4) as outp, \
                     tc.tile_pool(name="sps", bufs=4, space="PSUM") as sps, \
                     tc.tile_pool(name="yps", bufs=2, space="PSUM") as yps, \
                     tc.tile_pool(name="dps", bufs=2, space="PSUM") as dps:
                    if _rep == 0:
                        nc.sync.dma_start(wp_sb[:], wp_d)
                        nc.sync.dma_start(mk_sb[:],
                                          mk_d.rearrange("m p i -> p m i"))
                    def emit_outproj(yt, i0):
                        for tcc in range(4):
                            ost = outp.tile([128, 1024], F16, tag="ost")
                            for hh in range(2):
                                op2 = sps.tile([128, 512], F32, tag="spair")
                                nc.tensor.matmul(op2[:],
                                                 yt[:, ts(tcc, 128)],
                                                 wp_sb[:, ts(hh, 512)],
                                                 start=True, stop=True)
                                (nc.scalar.copy if hh == 0 else
                                 nc.vector.tensor_copy)(ost[:, ts(hh, 512)],
                                                        op2[:])
                            nc.sync.dma_start(
                                out_d[i0 + 128 * tcc:i0 + 128 * (tcc + 1), :],
                                ost[:])

                    pending = None
                    for b in range(NBLK):
                        i0 = TB * b
                        chunks = _chunk_list(b)
                        n = len(chunks)
                        yp = yps.tile([128, TB], F32, tag="y")
                        dp = dps.tile([128, TB], F32, tag="d")
                        LAG = 2
                        pts = {}
                        for step in range(n + LAG):
                            if step == 3 and pending is not None:
                                emit_outproj(*pending)
                                pending = None
                            if step < n:
                                j0, mi, lo, hi = chunks[step]
                                w = hi - lo
                                sp2 = sps.tile([128, 512], F32, tag="spair")
                                nc.tensor.matmul(sp2[:, 0:w],
                                                 kTn[:, j0:j0 + 128],
                                                 qTn[:, i0 + lo:i0 + hi],
                                                 start=True, stop=True)
                                pt = ptp.tile([128, 512], F16, tag="pt")
                                nc.scalar.activation(pt[:, 0:w], sp2[:, 0:w],
                                                     AF.Exp, scale=SCALE,
                                                     bias=eb[:])
                                if mi is not None:
                                    mlo = 128 * (mi if mi < 4 else mi - 4)
                                    psl = pt[:, mlo - lo:mlo - lo + 128]
                                    nc.vector.tensor_tensor(
                                        psl, psl, mk_sb[:, mi, mlo:mlo + 128], MUL)
                                pts[step] = pt
                            idx = step - LAG
                            if idx >= 0 and idx < n:
                                j0, mi, lo, hi = chunks[idx]
                                w = hi - lo
                                pt = pts.pop(idx)
                                st, sp_ = (idx == 0), (idx == n - 1)
                                nc.tensor.matmul(dp[:, lo:hi], on_sb[:],
                                                 pt[:, 0:w], start=st, stop=sp_)
                                nc.tensor.matmul(yp[:, lo:hi],
                                                 vsl[:, j0 // 128, :],
                                                 pt[:, 0:w], start=st, stop=sp_)
                        rc = sc2.tile([128, TB], F32, tag="rc")
                        nc.vector.reciprocal_approx_fast(rc[:], dp[:])
                        yt = sc2.tile([128, TB], F16, tag="yt")
                        nc.vector.tensor_tensor(yt[:], yp[:], rc[:], MUL)
                        pending = (yt, i0)
                    emit_outproj(*pending)

    nc.finalize()
    return nc


def _w_pre(w):
    # w_pre[p, co*128 + d] = w[co*128+p, d]
    return np.ascontiguousarray(
        w.reshape(NCO, 128, D).transpose(1, 0, 2).reshape(128, C)
    ).astype(np.float16)


def _build_masks():
    jj = np.arange(128)[:, None]
    ii = np.arange(512)[None, :]
    mk = np.zeros((8, 128, 512), dtype=np.float16)
    for m in range(4):
        mk[m] = (ii < jj + 128 * m).astype(np.float16)
    for m in range(4):
        mk[4 + m] = (ii >= jj + 128 * m).astype(np.float16)
    return mk


def kernel(x, ve, cos, sin, wq, wk, wv, w_gate, w_proj, window_size):
    from concourse.bass_utils import run_bass_kernel_spmd

    assert int(np.asarray(window_size)) == WIN
    x = np.asarray(x, dtype=np.float32)
    ve = np.asarray(ve, dtype=np.float32)
    cos = np.asarray(cos, dtype=np.float32).reshape(T, 64)
    sin = np.asarray(sin, dtype=np.float32).reshape(T, 64)
    wq = np.asarray(wq, dtype=np.float32)
    wk = np.asarray(wk, dtype=np.float32)
    wv = np.asarray(wv, dtype=np.float32)
    w_gate = np.asarray(w_gate, dtype=np.float32)
    w_proj = np.asarray(w_proj, dtype=np.float32)
    assert x.shape == (1, T, C) and ve.shape == (1, T, C)

    if "nc" not in _prog_cache:
        _prog_cache["nc"] = _build_program()
    nc = _prog_cache["nc"]

    # x_pre[p, co*T + t] = x[t, co*128+p]
    xT_h = np.ascontiguousarray(
        x[0].T.reshape(NCO, 128, T).transpose(1, 0, 2).reshape(128, NCO * T)
    ).astype(np.float16)
    cosT, sinT = cos.T, sin.T                                # [64, T]
    cc = np.concatenate([cosT, cosT], axis=0).astype(np.float16)
    # p[d] = u[d]*ssw[d]; y[d] = u[d]*cc[d] + p[swap(d)]
    # => ssw = [-sinT; sinT]
    ssw = np.concatenate([-sinT, sinT], axis=0).astype(np.float16)
    masks = _build_masks()
    ones = np.ones((128, 128), dtype=np.float16)

    in_maps = []
    for h in range(NCORES):
        d = D * h
        in_maps.append({
            "xT": xT_h,
            "cc": cc,
            "ssw": ssw,
            "veT": np.ascontiguousarray(2.0 * ve[0][:, d:d + D].T).astype(np.float16),
            "wq": _w_pre(wq[:, d:d + D]),
            "wk": _w_pre(wk[:, d:d + D]),
            "wv": _w_pre(wv[:, d:d + D]),
            "wp": np.ascontiguousarray(w_proj[d:d + D, :]).astype(np.float16),
            "wg": np.tile(w_gate[:, h:h + 1], (1, 128)).astype(np.float16),
            "masks": masks,
            "ones": ones,
        })

    global _last_in_maps
    _last_in_maps = in_maps
    res = run_bass_kernel_spmd(nc, in_maps, core_ids=list(range(NCORES)))
    out = np.zeros((T, C), dtype=np.float32)
    for h in range(NCORES):
        out += res.results[h]["out"].astype(np.float32)
    return out.reshape(1, T, C)

